# revision 2
# baseline (speedup 1.0000x reference)
"""LRFGraphConv Trainium2 kernel (v2).

Math: for each vertex i with neighbors N(i) (directed edge list, src=center):
    out[i] = ((sum_{j in N(i)} verts[j] - deg_i * verts[i]) @ lrf[i]) @ W.T + maxN * b

The neighbor-sum commutes with the per-center rotation and GEMM, so the
per-edge work collapses to a segment-sum of neighbor coordinates.  The
rotation and GEMM fuse into a single tensor-engine contraction over the 9
(j,k) pairs of u[i,(j,k)] = t[i,j]*lrf[i,j,k] against Wrep[(j,k),n] = W[n,k],
plus a constant-1 row carrying the maxN*b bias.  u uses 16 slots per vertex
(9 real + bias + 6 pad) so GEMM halves can anchor at partition 0/64.

Sharding: vertices are partitioned contiguously across 8 cores (6250 each).
The host buckets directed edges by owner of src, builds a per-core padded
neighbor table (NP=22 slots, zero padded), and gathers the halo neighbor
coordinates into it (the "halo exchange" done at shard time).  All per-core
inputs are packed into ONE dram blob loaded with 5 sliced DMAs (DMA issue
costs ~600ns of engine queue time each, so fewer+bigger wins).  Per chunk:
  DVE:    single 22-slot reduce + ~half the PSUM drain
  Pool:   u = t*lrf broadcast multiply; issues output stores
  PE:     transpose + GEMM (fp16), emitted back-to-back for HAM warmth
  Act:    uT copy + the other half of the PSUM drain
  Sync:   input loads
The last chunk is 2 tiles and flushed alone so the final store is small --
the NEFF teardown (semaphore-clear storm, ~6us, graded) starts right after
its completion.  No collectives.
"""

import os
import sys

sys.path.insert(0, "/opt/trn_rl_repo")

import numpy as np
import ml_dtypes

import concourse.bass as bass
import concourse.bacc as bacc
import concourse.tile as tile
from concourse import mybir
from concourse.masks import make_identity
from concourse.bass_utils import run_bass_kernel_spmd

V = 50000
NCORES = 8
VC = V // NCORES          # 6250 owned vertices per core
P = 128
NVT = (VC + P - 1) // P   # 49 vertex tiles per core
VCP = NVT * P             # 6272 padded
NP = 22                   # neighbor slots (last = -deg*verts fold slot)
MAXNV = 8                 # tiles per chunk (8*16 = 128 partitions)


def make_chunks(nbt):
    """First chunks tapered for fast pipeline fill; the LAST chunk holds the
    tier-B (overflow) tiles and is small so the final store (which gates the
    graded teardown) is tiny."""
    last = max(nbt, 2)
    ch = [4, 6]
    rem = NVT - sum(ch) - last
    while rem > MAXNV:
        ch.append(MAXNV)
        rem -= MAXNV
    if rem:
        ch.append(rem)
    ch.append(last)
    assert sum(ch) == NVT and all(1 <= x <= MAXNV for x in ch)
    return ch


BF = mybir.dt.float16
BF_NP = np.float16

LAST_RESULTS = None       # BassKernelResults of the most recent run (for test.py)


def build(nc: bass.Bass, NBT: int, NPB: int, CHUNKS):
    dt = mybir.dt
    NCH = len(CHUNKS)
    vstart = [0]
    for nv in CHUNKS:
        vstart.append(vstart[-1] + nv)

    # ---- packed input blob layout (fp16 cols per partition row) ----
    # [ wr 512 | aux NVT*9 | xp c0 | xp c1 | ... | xp c{n-1} | xpb ]
    W_COLS = 512
    AUX_COLS = NVT * 9
    off = 0
    o_wr = off; off += W_COLS
    o_aux = off; off += AUX_COLS
    o_xp = []
    for nv in CHUNKS:
        o_xp.append(off)
        off += nv * 3 * NP
    o_xpb = off
    if NBT > 0:
        off += NBT * 3 * NPB
    TOT = off

    blob = nc.dram_tensor("blob", [P, TOT], BF, kind="ExternalInput")
    out = nc.dram_tensor("out", [P, NVT * P], dt.float16, kind="ExternalOutput")

    # load groups: (start_col, end_col, chunks covered)
    groups = [(0, o_xp[0] + CHUNKS[0] * 3 * NP, [0])]
    ci = 1
    while ci < NCH:
        cj = min(ci + 2, NCH)
        if cj == NCH:          # last group also covers xpb
            groups.append((o_xp[ci], TOT, list(range(ci, cj))))
        else:
            groups.append((o_xp[ci], o_xp[cj - 1] + CHUNKS[cj - 1] * 3 * NP,
                           list(range(ci, cj))))
        ci = cj

    with tile.TileContext(nc) as tc:
        with (
            tc.tile_pool(name="c", bufs=1) as cpool,
            tc.tile_pool(name="x", bufs=3) as xpool,
            tc.tile_pool(name="w", bufs=4) as wpool,
            tc.tile_pool(name="pt", bufs=2, space="PSUM") as pst,
            tc.tile_pool(name="pg", bufs=3, space="PSUM") as psg,
        ):
            outsb = cpool.tile([P, NVT * P], dt.float16)
            ident = cpool.tile([P, P], BF)
            with tc.high_priority():
                make_identity(nc, ident[:])
            # one persistent SBUF strip holding wr + aux + per-chunk tables
            xsb = cpool.tile([P, TOT], BF, tag="xsb")
            w_t = xsb[:, o_wr : o_wr + W_COLS]
            auxfull = xsb[:, o_aux : o_aux + AUX_COLS].rearrange(
                "p (v f) -> p v f", f=9
            )
            gload = [None] * NCH  # chunk -> its load group index (for deps)
            for gi, (lo, hi, chs) in enumerate(groups):
                nc.sync.dma_start(out=xsb[:, lo:hi], in_=blob[:, lo:hi])
                for c in chs:
                    gload[c] = gi

            # persistent u tiles (4-deep rotation); bias slot 9 = 1, 10:16 = 0
            u_bufs = []
            for s in range(4):
                ub = cpool.tile([P, MAXNV * 16], BF, tag=f"u{s}")
                nc.gpsimd.memset(ub[:], 0.0)
                nc.gpsimd.memset(
                    ub[:].rearrange("p (v s) -> p v s", s=16)[:, :, 9:10], 1.0
                )
                u_bufs.append(ub)

            state = [None] * NCH  # per-chunk (u, uT, pg)

            def stage_reduce_mult(c):
                nv = CHUNKS[c]
                vlo = vstart[c]
                xv = xsb[:, o_xp[c] : o_xp[c] + nv * 3 * NP].rearrange(
                    "p (v c n) -> p v c n", v=nv, c=3, n=NP
                )
                # t = sum over all NP slots (one holds -deg*verts)
                t = wpool.tile([P, MAXNV * 3], BF, tag="t")
                with nc.allow_low_precision(reason="fp16 neighbor sums"):
                    nc.vector.tensor_reduce(
                        out=t[:, : nv * 3], in_=xv,
                        axis=mybir.AxisListType.X,
                        op=mybir.AluOpType.add,
                    )
                if NBT > 0 and c == NCH - 1:
                    # overflow slots of high-degree verts (the last NBT tiles)
                    xb = xsb[:, o_xpb : o_xpb + NBT * 3 * NPB]
                    tB = wpool.tile([P, MAXNV * 3], BF, tag="t")
                    with nc.allow_low_precision(reason="fp16 neighbor sums"):
                        nc.vector.tensor_reduce(
                            out=tB[:, : NBT * 3],
                            in_=xb.rearrange(
                                "p (v c n) -> p v c n", v=NBT, c=3, n=NPB
                            ),
                            axis=mybir.AxisListType.X,
                            op=mybir.AluOpType.add,
                        )
                    nc.vector.tensor_tensor(
                        out=t[:, (nv - NBT) * 3 : nv * 3],
                        in0=t[:, (nv - NBT) * 3 : nv * 3],
                        in1=tB[:, : NBT * 3],
                        op=mybir.AluOpType.add,
                    )

                # u[p, v, k*3+j] = t[p,v,j]*lrf9[p,v,k*3+j] broadcast mul (Pool)
                u = u_bufs[c % 4]
                u9 = u[:, : nv * 16].rearrange("p (v s) -> p v s", s=16)[
                    :, :, 0:9
                ].rearrange("p v (k j) -> p v k j", k=3, j=3)
                aux9 = auxfull[:, vlo : vlo + nv, :]
                t4 = t[:, : nv * 3].rearrange("p (v c) -> p v c", c=3).unsqueeze(2)
                nc.gpsimd.tensor_tensor(
                    out=u9,
                    in0=t4.to_broadcast([P, nv, 3, 3]),
                    in1=aux9.rearrange("p v (k j) -> p v k j", k=3, j=3),
                    op=mybir.AluOpType.mult,
                )
                state[c] = [u, None, None]

            def stage_tu(c):
                nv = CHUNKS[c]
                cw = nv * 16
                u = state[c][0]
                pt = pst.tile([P, P], BF, tag="pt")
                nc.tensor.transpose(
                    out=pt[:cw, :], in_=u[:, :cw], identity=ident[:]
                )
                uT = wpool.tile([P, P], BF, tag="uT")
                # alternate the uT drain between Scalar and Vector to balance
                if c % 2 == 0:
                    nc.scalar.copy(out=uT[:cw, :], in_=pt[:cw, :])
                else:
                    nc.vector.tensor_copy(out=uT[:cw, :], in_=pt[:cw, :])
                state[c][1] = uT

            def stage_gemm(c):
                nv = CHUNKS[c]
                uT = state[c][1]
                pg = psg.tile([P, MAXNV * P], dt.float32, tag="pg")
                g = 0
                while g < nv:
                    ng = min(4, nv - g)
                    rb = 16 * g
                    nc.tensor.matmul(
                        out=pg[:, g * P : (g + ng) * P],
                        lhsT=uT[rb : rb + 16 * ng, :],
                        rhs=w_t[rb : rb + 16 * ng, : ng * P],
                        start=True,
                        stop=True,
                    )
                    g += ng
                state[c][2] = pg

            def stage_drain_store(c):
                nv = CHUNKS[c]
                ow = nv * P
                olo = vstart[c] * P
                pg = state[c][2]
                # split the PSUM drain between Vector and Scalar
                nsp = (ow // 2) // P * P
                if nsp == 0:
                    nsp = min(P, ow)
                nc.vector.tensor_copy(
                    out=outsb[:, olo : olo + nsp], in_=pg[:, :nsp]
                )
                if ow > nsp:
                    nc.scalar.copy(
                        out=outsb[:, olo + nsp : olo + ow], in_=pg[:, nsp:ow]
                    )
                # flush policy: pairs, but the final chunk flushes alone so the
                # last store (which gates the teardown) is small
                flush_after = {1: (0, 2), 3: (2, 4), 5: (4, 6),
                               NCH - 2: (NCH - 2 if NCH - 2 > 6 else 6, NCH - 1),
                               NCH - 1: (NCH - 1, NCH)}
                if c in flush_after and (c != NCH - 2 or NCH - 2 > 5):
                    c0, c1 = flush_after[c]
                    lo = vstart[c0] * P
                    hi = vstart[c1] * P
                    nc.gpsimd.dma_start(out=out[:, lo:hi], in_=outsb[:, lo:hi])
                elif c == NCH - 2:
                    lo = vstart[NCH - 2] * P
                    hi = vstart[NCH - 1] * P
                    nc.gpsimd.dma_start(out=out[:, lo:hi], in_=outsb[:, lo:hi])

            # software-pipelined emission: reduce/mult 2 chunks ahead of the
            # GEMM, transpose/uT 1 ahead, drain right after its GEMM
            for i in range(NCH + 3):
                if i < NCH:
                    stage_reduce_mult(i)
                if 0 <= i - 1 < NCH:
                    stage_tu(i - 1)
                if 0 <= i - 2 < NCH:
                    stage_gemm(i - 2)
                if 0 <= i - 3 < NCH:
                    stage_drain_store(i - 3)
    return nc


def _host_prep(verts, edges, lrf, W, b):
    vb = np.asarray(verts, dtype=np.float32)
    e = np.asarray(edges).astype(np.int64)
    src = np.concatenate([e[:, 0], e[:, 1]]).astype(np.int64)
    dst = np.concatenate([e[:, 1], e[:, 0]]).astype(np.int64)

    deg = np.bincount(src, minlength=V).astype(np.int64)
    maxN = int(deg.max())
    # two-tier: main table has NP slots (last = fold); deg > NP-1 vertices are
    # remapped to the trailing v-tiles and spill into the overflow table.
    CAP = NP - 1
    over = (deg > CAP).reshape(NCORES, VC)
    nB = over.sum(axis=1)
    NBT = (
        int(max(NVT - (VC - int(n)) // P for n in nB)) if maxN > CAP else 0
    )
    NPB = max(0, ((maxN - CAP + 3) // 4) * 4)

    # per-core remap: overflow verts go last (their overflow reduce runs in
    # the pipeline tail)
    newpos = np.empty((NCORES, VC), np.int64)
    order_c = np.empty((NCORES, VC), np.int64)
    for cc in range(NCORES):
        oc = np.concatenate([np.where(~over[cc])[0], np.where(over[cc])[0]])
        order_c[cc] = oc
        newpos[cc, oc] = np.arange(VC)

    order = np.argsort(src, kind="stable")
    src_s = src[order]
    dst_s = dst[order]
    starts = np.zeros(V + 1, np.int64)
    np.cumsum(deg, out=starts[1:])
    slot = np.arange(src_s.size, dtype=np.int64) - starts[src_s]

    c_a = src_s // VC
    il_new = newpos[c_a, src_s - c_a * VC]
    p_a = il_new % P
    v_a = il_new // P
    vals = vb[dst_s].astype(BF_NP)

    Xp = np.zeros((NCORES, P, NVT, 3, NP), BF_NP)
    inA = slot < CAP
    Xp[c_a[inA], p_a[inA], v_a[inA], :, slot[inA]] = vals[inA]
    if NBT > 0:
        XpB = np.zeros((NCORES, P, NBT, 3, NPB), BF_NP)
        inB = ~inA
        XpB[c_a[inB], p_a[inB], v_a[inB] - (NVT - NBT), :, slot[inB] - CAP] = vals[inB]
    else:
        XpB = np.zeros((NCORES, P, 0, 3, 0), BF_NP)

    # fold slot: -deg*verts for the owned vertex goes in the last A slot
    dv = (-deg[:, None].astype(np.float32)) * vb
    dv_pad = np.zeros((NCORES, VCP, 3), np.float32)
    for cc in range(NCORES):
        dv_pad[cc, :VC] = dv.reshape(NCORES, VC, 3)[cc][order_c[cc]]
    Xp[:, :, :, :, NP - 1] = dv_pad.reshape(NCORES, NVT, P, 3).transpose(
        0, 2, 1, 3
    ).astype(BF_NP)

    # aux per vertex: lrf(9), remapped -> [NC, P, NVT*9]
    aux_flat = np.zeros((NCORES, VCP, 9), np.float32)
    # k-major flattening: slot s = k*3+j holds lrf[:, j, k]
    lrf9 = np.ascontiguousarray(
        np.asarray(lrf, np.float32).reshape(NCORES, VC, 3, 3).transpose(0, 1, 3, 2)
    ).reshape(NCORES, VC, 9)
    for cc in range(NCORES):
        aux_flat[cc, :VC] = lrf9[cc][order_c[cc]]
    auxh = np.ascontiguousarray(
        aux_flat.reshape(NCORES, NVT, P, 9).transpose(0, 2, 1, 3)
    ).reshape(NCORES, P, NVT * 9).astype(BF_NP)

    Wf = np.asarray(W, np.float32)
    W16 = np.zeros((16, P), np.float32)
    for s in range(9):
        W16[s, :] = Wf[:, s // 3]   # k-major: slot s = k*3+j -> k = s//3
    W16[9, :] = maxN * np.asarray(b, np.float32)
    # Block-diagonal [128, 512]: 4 column blocks of W16, replicated in both
    # 64-row halves so matmuls can anchor at partition 0 or 64.
    half = np.zeros((64, 512), np.float32)
    for q in range(4):
        half[16 * q : 16 * q + 16, 128 * q : 128 * q + 128] = W16
    Wr = np.ascontiguousarray(np.vstack([half, half])).astype(BF_NP)

    CH = make_chunks(NBT)
    in_maps = []
    for c in range(NCORES):
        xpf = Xp[c].reshape(P, NVT, 3 * NP)
        parts = [Wr, np.ascontiguousarray(auxh[c])]
        vlo = 0
        for nv in CH:
            parts.append(
                np.ascontiguousarray(xpf[:, vlo : vlo + nv].reshape(P, nv * 3 * NP))
            )
            vlo += nv
        if NBT > 0:
            parts.append(np.ascontiguousarray(XpB[c].reshape(P, NBT * 3 * NPB)))
        blob = np.concatenate(parts, axis=1)
        in_maps.append({"blob": np.ascontiguousarray(blob)})
    return in_maps, NBT, NPB, CH, order_c


def kernel(verts, edges, lrf, W, b):
    global LAST_RESULTS
    in_maps, NBT, NPB, CH, order_c = _host_prep(verts, edges, lrf, W, b)

    nc = bacc.Bacc()
    build(nc, NBT, NPB, CH)
    nc.finalize()

    trace = os.environ.get("KBENCH_TRACE") == "1"
    res = run_bass_kernel_spmd(
        nc, in_maps, core_ids=list(range(NCORES)), trace=trace
    )
    LAST_RESULTS = res

    full = np.empty((V, 128), np.float32)
    for c in range(NCORES):
        o = (
            res.results[c]["out"].astype(np.float32)
            .reshape(P, NVT, P).transpose(1, 0, 2).reshape(VCP, P)[:VC]
        )
        blk = full[c * VC : (c + 1) * VC]
        blk[order_c[c]] = o
    return full


# revision 3
# speedup vs baseline: 1.0462x; 1.0462x over previous
"""LRFGraphConv Trainium2 kernel (v3).

Math: for each vertex i with neighbors N(i) (directed edge list, src=center):
    out[i] = ((sum_{j in N(i)} verts[j] - deg_i * verts[i]) @ lrf[i]) @ W.T + maxN * b

The neighbor-sum commutes with the per-center rotation and GEMM, so the
per-edge work collapses to a segment-sum of neighbor coordinates.  The
rotation and GEMM fuse into a single tensor-engine contraction over the 9
(j,k) pairs of u[i,(j,k)] = t[i,j]*lrf[i,j,k] against Wrep[(j,k),n] = W[n,k],
plus a constant-1 row carrying the maxN*b bias.  u uses 16 slots per vertex
(9 real + bias + 6 pad) so GEMM halves can anchor at partition 0/64.

Sharding: vertices are partitioned contiguously across 8 cores (6250 each),
then sorted by degree (ascending) within each core.  The host buckets
directed edges by owner of src and builds per-chunk padded neighbor tables
whose slot count is that chunk's max degree + 1 (the "+1" fold slot holds
-deg*verts) -- low-degree chunks get narrow tables, so table bytes and
reduce work drop ~20% vs a uniform cap, and no overflow tier is needed.
All inputs are packed into ONE dram blob loaded with 6 sliced DMAs (each
DMA issue costs ~600ns of engine queue time).  Per chunk:
  DVE:    per-chunk slot reduce + 3/8 of the PSUM drain
  Pool:   u = t*lrf broadcast multiply; issues output stores
  PE:     transpose + GEMM (fp16), emitted back-to-back for HAM warmth
  Act:    uT copy + 5/8 of the PSUM drain
  Sync:   input loads
The last chunk is 2 tiles (the highest-degree verts) and flushed alone so
the final store -- which gates the graded NEFF teardown (semaphore-clear
storm, ~6us) -- is small.  No collectives.
"""

import os
import sys

sys.path.insert(0, "/opt/trn_rl_repo")

import numpy as np

import concourse.bass as bass
import concourse.bacc as bacc
import concourse.tile as tile
from concourse import mybir
from concourse.masks import make_identity
from concourse.bass_utils import run_bass_kernel_spmd

V = 50000
NCORES = 8
VC = V // NCORES          # 6250 owned vertices per core
P = 128
NVT = (VC + P - 1) // P   # 49 vertex tiles per core
VCP = NVT * P             # 6272 padded
MAXNV = 8                 # tiles per chunk (PSUM: 8*128 fp32 = 2 banks)

CHUNKS = [4, 6, 8, 8, 8, 8, 5, 2]
assert sum(CHUNKS) == NVT

BF = mybir.dt.float16
BF_NP = np.float16

LAST_RESULTS = None       # BassKernelResults of the most recent run (for test.py)


def build(nc: bass.Bass, NPC):
    """NPC[c] = slot count (max degree + 1 fold slot) for chunk c."""
    dt = mybir.dt
    NCH = len(CHUNKS)
    vstart = [0]
    for nv in CHUNKS:
        vstart.append(vstart[-1] + nv)

    # ---- packed input blob layout (fp16 cols per partition row) ----
    # [ xp c0 | wr 512 | aux NVT*9 | xp c1 | ... | xp c7 ]
    W_COLS = 512
    AUX_COLS = NVT * 9
    off = 0
    o_xp = [off]
    off += CHUNKS[0] * 3 * NPC[0]
    o_wr = off; off += W_COLS
    o_aux = off; off += AUX_COLS
    for c in range(1, NCH):
        o_xp.append(off)
        off += CHUNKS[c] * 3 * NPC[c]
    TOT = off

    blob = nc.dram_tensor("blob", [P, TOT], BF, kind="ExternalInput")
    out = nc.dram_tensor("out", [P, NVT * P], dt.float16, kind="ExternalOutput")

    # load groups (start_col, end_col): c0 first for fast pipeline fill
    groups = [
        (0, o_xp[0] + CHUNKS[0] * 3 * NPC[0]),          # c0
        (o_wr, o_aux + AUX_COLS),                        # wr + aux
        (o_xp[1], o_xp[2] + CHUNKS[2] * 3 * NPC[2]),     # c1 c2
        (o_xp[3], o_xp[4] + CHUNKS[4] * 3 * NPC[4]),     # c3 c4
        (o_xp[5], o_xp[6] + CHUNKS[6] * 3 * NPC[6]),     # c5 c6
        (o_xp[7], TOT),                                  # c7
    ]

    with tile.TileContext(nc) as tc:
        with (
            tc.tile_pool(name="c", bufs=1) as cpool,
            tc.tile_pool(name="w", bufs=4) as wpool,
            tc.tile_pool(name="pt", bufs=2, space="PSUM") as pst,
            tc.tile_pool(name="pg", bufs=3, space="PSUM") as psg,
        ):
            outsb = cpool.tile([P, NVT * P], dt.float16)
            ident = cpool.tile([P, P], BF)
            with tc.high_priority():
                make_identity(nc, ident[:])
            xsb = cpool.tile([P, TOT], BF, tag="xsb")
            w_t = xsb[:, o_wr : o_wr + W_COLS]
            auxfull = xsb[:, o_aux : o_aux + AUX_COLS].rearrange(
                "p (v f) -> p v f", f=9
            )
            for lo, hi in groups:
                nc.sync.dma_start(out=xsb[:, lo:hi], in_=blob[:, lo:hi])

            # persistent u tiles (4-deep rotation); bias slot 9 = 1, 10:16 = 0
            u_bufs = []
            for s in range(4):
                ub = cpool.tile([P, MAXNV * 16], BF, tag=f"u{s}")
                nc.gpsimd.memset(ub[:], 0.0)
                nc.gpsimd.memset(
                    ub[:].rearrange("p (v s) -> p v s", s=16)[:, :, 9:10], 1.0
                )
                u_bufs.append(ub)

            state = [None] * NCH  # per-chunk (u, uT, pg)

            def stage_reduce_mult(c):
                nv = CHUNKS[c]
                npc = NPC[c]
                vlo = vstart[c]
                xv = xsb[:, o_xp[c] : o_xp[c] + nv * 3 * npc].rearrange(
                    "p (v c n) -> p v c n", v=nv, c=3, n=npc
                )
                t = wpool.tile([P, MAXNV * 3], BF, tag="t")
                with nc.allow_low_precision(reason="fp16 neighbor sums"):
                    nc.vector.tensor_reduce(
                        out=t[:, : nv * 3], in_=xv,
                        axis=mybir.AxisListType.X,
                        op=mybir.AluOpType.add,
                    )
                # u[p, v, k*3+j] = t[p,v,j]*lrf9[p,v,k*3+j] broadcast mul (Pool)
                u = u_bufs[c % 4]
                u9 = u[:, : nv * 16].rearrange("p (v s) -> p v s", s=16)[
                    :, :, 0:9
                ].rearrange("p v (k j) -> p v k j", k=3, j=3)
                aux9 = auxfull[:, vlo : vlo + nv, :]
                t4 = t[:, : nv * 3].rearrange("p (v c) -> p v c", c=3).unsqueeze(2)
                nc.gpsimd.tensor_tensor(
                    out=u9,
                    in0=t4.to_broadcast([P, nv, 3, 3]),
                    in1=aux9.rearrange("p v (k j) -> p v k j", k=3, j=3),
                    op=mybir.AluOpType.mult,
                )
                state[c] = [u, None, None]

            def stage_tu(c):
                nv = CHUNKS[c]
                cw = nv * 16
                u = state[c][0]
                pt = pst.tile([P, P], BF, tag="pt")
                nc.tensor.transpose(
                    out=pt[:cw, :], in_=u[:, :cw], identity=ident[:]
                )
                uT = wpool.tile([P, P], BF, tag="uT")
                nc.scalar.copy(out=uT[:cw, :], in_=pt[:cw, :])
                state[c][1] = uT

            def stage_gemm(c):
                nv = CHUNKS[c]
                uT = state[c][1]
                pg = psg.tile([P, MAXNV * P], dt.float32, tag="pg")
                g = 0
                while g < nv:
                    ng = min(4, nv - g)
                    rb = 16 * g
                    nc.tensor.matmul(
                        out=pg[:, g * P : (g + ng) * P],
                        lhsT=uT[rb : rb + 16 * ng, :],
                        rhs=w_t[rb : rb + 16 * ng, : ng * P],
                        start=True,
                        stop=True,
                    )
                    g += ng
                state[c][2] = pg

            # store flush groups: pairs, then the last two chunks alone
            flush_of = {}
            for c0f, c1f in [(0, 1), (2, 3), (4, 5), (6, 6), (7, 7)]:
                flush_of[c1f] = (c0f, c1f + 1)

            def stage_drain_store(c):
                nv = CHUNKS[c]
                ow = nv * P
                olo = vstart[c] * P
                pg = state[c][2]
                # split the PSUM drain: 3/8 Vector, 5/8 Scalar
                nsp = (3 * ow // 8) // P * P
                if nsp == 0:
                    nsp = min(P, ow)
                nc.vector.tensor_copy(
                    out=outsb[:, olo : olo + nsp], in_=pg[:, :nsp]
                )
                if ow > nsp:
                    nc.scalar.copy(
                        out=outsb[:, olo + nsp : olo + ow], in_=pg[:, nsp:ow]
                    )
                if c in flush_of:
                    ca, cb = flush_of[c]
                    lo = vstart[ca] * P
                    hi = vstart[cb] * P
                    nc.gpsimd.dma_start(out=out[:, lo:hi], in_=outsb[:, lo:hi])

            # software-pipelined emission: reduce/mult 2 chunks ahead of the
            # GEMM, transpose/uT 1 ahead, drain right after its GEMM
            for i in range(NCH + 3):
                if i < NCH:
                    stage_reduce_mult(i)
                if 0 <= i - 1 < NCH:
                    stage_tu(i - 1)
                if 0 <= i - 2 < NCH:
                    stage_gemm(i - 2)
                if 0 <= i - 3 < NCH:
                    stage_drain_store(i - 3)
    return nc


def _host_prep(verts, edges, lrf, W, b):
    vb = np.asarray(verts, dtype=np.float32)
    e = np.asarray(edges).astype(np.int64)
    src = np.concatenate([e[:, 0], e[:, 1]]).astype(np.int64)
    dst = np.concatenate([e[:, 1], e[:, 0]]).astype(np.int64)

    deg = np.bincount(src, minlength=V).astype(np.int64)
    maxN = int(deg.max())

    # per-core remap: sort by degree ascending -> low-degree chunks get
    # narrow neighbor tables
    degc = deg.reshape(NCORES, VC)
    newpos = np.empty((NCORES, VC), np.int64)
    order_c = np.empty((NCORES, VC), np.int64)
    for cc in range(NCORES):
        oc = np.argsort(degc[cc], kind="stable")
        order_c[cc] = oc
        newpos[cc, oc] = np.arange(VC)

    # per-chunk slot counts NPC[c] = max degree in chunk + 1 (fold slot),
    # padded so every vertex position in the padded tile range is covered
    NCH = len(CHUNKS)
    vstart = [0]
    for nv in CHUNKS:
        vstart.append(vstart[-1] + nv)
    deg_sorted = np.sort(degc, axis=1)          # per core, ascending
    NPC = []
    for c in range(NCH):
        hi = min(vstart[c + 1] * P, VC)
        cap = int(deg_sorted[:, :hi].max())     # max over cores for SPMD
        NPC.append(cap + 1)

    order = np.argsort(src, kind="stable")
    src_s = src[order]
    dst_s = dst[order]
    starts = np.zeros(V + 1, np.int64)
    np.cumsum(deg, out=starts[1:])
    slot = np.arange(src_s.size, dtype=np.int64) - starts[src_s]

    c_a = src_s // VC
    il_new = newpos[c_a, src_s - c_a * VC]
    p_a = il_new % P
    v_a = il_new // P
    vals = vb[dst_s].astype(BF_NP)

    # chunk id per table entry
    chunk_of_tile = np.zeros(NVT, np.int64)
    for c in range(NCH):
        chunk_of_tile[vstart[c] : vstart[c + 1]] = c
    ch_a = chunk_of_tile[v_a]

    # per-chunk tables [P, nv, 3, NPC[c]]
    Xp = [np.zeros((NCORES, P, CHUNKS[c], 3, NPC[c]), BF_NP) for c in range(NCH)]
    for c in range(NCH):
        m = ch_a == c
        Xp[c][c_a[m], p_a[m], v_a[m] - vstart[c], :, slot[m]] = vals[m]

    # fold slot: -deg*verts for the owned vertex goes in the last slot
    dv = (-deg[:, None].astype(np.float32)) * vb
    dv_pad = np.zeros((NCORES, VCP, 3), np.float32)
    for cc in range(NCORES):
        dv_pad[cc, :VC] = dv.reshape(NCORES, VC, 3)[cc][order_c[cc]]
    dv_t = dv_pad.reshape(NCORES, NVT, P, 3).transpose(0, 2, 1, 3)  # [NC,P,NVT,3]
    for c in range(NCH):
        Xp[c][:, :, :, :, NPC[c] - 1] = dv_t[
            :, :, vstart[c] : vstart[c + 1], :
        ].astype(BF_NP)

    # aux per vertex: lrf(9), remapped -> [NC, P, NVT*9]
    aux_flat = np.zeros((NCORES, VCP, 9), np.float32)
    # k-major flattening: slot s = k*3+j holds lrf[:, j, k]
    lrf9 = np.ascontiguousarray(
        np.asarray(lrf, np.float32).reshape(NCORES, VC, 3, 3).transpose(0, 1, 3, 2)
    ).reshape(NCORES, VC, 9)
    for cc in range(NCORES):
        aux_flat[cc, :VC] = lrf9[cc][order_c[cc]]
    auxh = np.ascontiguousarray(
        aux_flat.reshape(NCORES, NVT, P, 9).transpose(0, 2, 1, 3)
    ).reshape(NCORES, P, NVT * 9).astype(BF_NP)

    Wf = np.asarray(W, np.float32)
    W16 = np.zeros((16, P), np.float32)
    for s in range(9):
        W16[s, :] = Wf[:, s // 3]   # k-major: slot s = k*3+j -> k = s//3
    W16[9, :] = maxN * np.asarray(b, np.float32)
    # Block-diagonal [128, 512]: 4 column blocks of W16, replicated in both
    # 64-row halves so matmuls can anchor at partition 0 or 64.
    half = np.zeros((64, 512), np.float32)
    for q in range(4):
        half[16 * q : 16 * q + 16, 128 * q : 128 * q + 128] = W16
    Wr = np.ascontiguousarray(np.vstack([half, half])).astype(BF_NP)

    in_maps = []
    for cc in range(NCORES):
        parts = [np.ascontiguousarray(Xp[0][cc].reshape(P, -1)), Wr,
                 np.ascontiguousarray(auxh[cc])]
        for c in range(1, NCH):
            parts.append(np.ascontiguousarray(Xp[c][cc].reshape(P, -1)))
        in_maps.append({"blob": np.ascontiguousarray(np.concatenate(parts, axis=1))})
    return in_maps, NPC, order_c


def kernel(verts, edges, lrf, W, b):
    global LAST_RESULTS
    in_maps, NPC, order_c = _host_prep(verts, edges, lrf, W, b)

    nc = bacc.Bacc()
    build(nc, NPC)
    nc.finalize()

    trace = os.environ.get("KBENCH_TRACE") == "1"
    res = run_bass_kernel_spmd(
        nc, in_maps, core_ids=list(range(NCORES)), trace=trace
    )
    LAST_RESULTS = res

    full = np.empty((V, 128), np.float32)
    for c in range(NCORES):
        o = (
            res.results[c]["out"].astype(np.float32)
            .reshape(P, NVT, P).transpose(1, 0, 2).reshape(VCP, P)[:VC]
        )
        blk = full[c * VC : (c + 1) * VC]
        blk[order_c[c]] = o
    return full


# revision 4
# speedup vs baseline: 1.1214x; 1.0719x over previous
"""LRFGraphConv Trainium2 kernel (v4).

Math: for each vertex i with neighbors N(i) (directed edge list, src=center):
    out[i] = ((sum_{j in N(i)} verts[j] - deg_i * verts[i]) @ lrf[i]) @ W.T + maxN * b

The neighbor-sum commutes with the per-center rotation and GEMM, so the
per-edge work collapses to a segment-sum of neighbor coordinates.  The
rotation and GEMM fuse into a single tensor-engine contraction over the 9
(j,k) pairs of u[i,(j,k)] = t[i,j]*lrf[i,j,k] against Wrep[(j,k),n] = W[n,k],
plus a constant-1 row carrying the maxN*b bias.  u uses 16 slots per vertex
(9 real + bias + 6 pad) so GEMM halves can anchor at partition 0/64.

Sharding: vertices are partitioned contiguously across 8 cores (6250 each),
then sorted by degree (ascending) within each core.  The host buckets
directed edges by owner of src and builds per-chunk padded neighbor tables
whose slot count is that chunk's max degree + 1 (the "+1" fold slot holds
-deg*verts) -- low-degree chunks get narrow tables, so table bytes and
reduce work drop ~20% vs a uniform cap, and no overflow tier is needed.
Each load group carries its own lrf slice so the first multiply is never
gated on a big aux transfer.  Engine assignment per chunk:
  DVE:    per-chunk slot reduce + uT (transpose PSUM->SBUF) copies
  Pool:   u = t*lrf broadcast multiply; issues the tail output stores
  PE:     transpose + GEMM (fp16)
  Act:    all PSUM output drains (fp32->fp16)
  Sync:   input loads + the first two output stores
The last chunk is 2 tiles (the highest-degree verts) and flushed alone so
the final store -- which gates the graded NEFF teardown (semaphore-clear
storm + barriers, ~7.7us fixed) -- is small.  No collectives.
"""

import os
import sys

sys.path.insert(0, "/opt/trn_rl_repo")

import numpy as np

import concourse.bass as bass
import concourse.bacc as bacc
import concourse.tile as tile
from concourse import mybir
from concourse.masks import make_identity
from concourse.bass_utils import run_bass_kernel_spmd

V = 50000
NCORES = 8
VC = V // NCORES          # 6250 owned vertices per core
P = 128
NVT = (VC + P - 1) // P   # 49 vertex tiles per core
VCP = NVT * P             # 6272 padded
MAXNV = 8                 # tiles per chunk (PSUM: 8*128 fp32 = 2 banks)

CHUNKS = [4, 6, 8, 8, 8, 8, 5, 2]
assert sum(CHUNKS) == NVT
NCH = len(CHUNKS)
# load groups: chunks loaded together (each group also carries its aux slice)
LGROUPS = [[0], [1, 2], [3, 4], [5, 6], [7]]

BF = mybir.dt.float16
BF_NP = np.float16

LAST_RESULTS = None       # BassKernelResults of the most recent run (for test.py)


def build(nc: bass.Bass, NPC):
    """NPC[c] = slot count (max degree + 1 fold slot) for chunk c."""
    dt = mybir.dt
    vstart = [0]
    for nv in CHUNKS:
        vstart.append(vstart[-1] + nv)

    # ---- packed input blob layout (fp16 cols per partition row) ----
    # group 0 tables+aux | wr | group 1 tables+aux | group 2 ... |
    W_COLS = 512
    o_xp = [0] * NCH
    o_aux = [0] * NCH
    off = 0
    group_span = []
    for gi, chs in enumerate(LGROUPS):
        glo = off
        for c in chs:
            o_xp[c] = off
            off += CHUNKS[c] * 3 * NPC[c]
        for c in chs:
            o_aux[c] = off
            off += CHUNKS[c] * 9
        group_span.append((glo, off))
        if gi == 0:
            o_wr = off
            off += W_COLS
    TOT = off

    blob = nc.dram_tensor("blob4", [P, TOT], BF, kind="ExternalInput")
    out = nc.dram_tensor("out", [P, NVT * P], dt.float16, kind="ExternalOutput")

    with tile.TileContext(nc) as tc:
        with (
            tc.tile_pool(name="c", bufs=1) as cpool,
            tc.tile_pool(name="w", bufs=4) as wpool,
            tc.tile_pool(name="pt", bufs=2, space="PSUM") as pst,
            tc.tile_pool(name="pg", bufs=3, space="PSUM") as psg,
        ):
            outsb = cpool.tile([P, NVT * P], dt.float16)
            ident = cpool.tile([P, P], BF)
            with tc.high_priority():
                make_identity(nc, ident[:])
            xsb = cpool.tile([P, TOT], BF, tag="xsb")
            w_t = xsb[:, o_wr : o_wr + W_COLS]
            # loads: group 0, then wr, then the rest
            nc.sync.dma_start(
                out=xsb[:, group_span[0][0] : group_span[0][1]],
                in_=blob[:, group_span[0][0] : group_span[0][1]],
            )
            nc.sync.dma_start(
                out=xsb[:, o_wr : o_wr + W_COLS], in_=blob[:, o_wr : o_wr + W_COLS]
            )
            for lo, hi in group_span[1:]:
                nc.sync.dma_start(out=xsb[:, lo:hi], in_=blob[:, lo:hi])

            # persistent u tiles (4-deep rotation); bias slot 9 = 1, 10:16 = 0
            u_bufs = []
            for s in range(4):
                ub = cpool.tile([P, MAXNV * 16], BF, tag=f"u{s}")
                nc.gpsimd.memset(ub[:], 0.0)
                nc.gpsimd.memset(
                    ub[:].rearrange("p (v s) -> p v s", s=16)[:, :, 9:10], 1.0
                )
                u_bufs.append(ub)

            state = [None] * NCH  # per-chunk (u, uT, pg)

            def stage_reduce_mult(c):
                nv = CHUNKS[c]
                npc = NPC[c]
                xv = xsb[:, o_xp[c] : o_xp[c] + nv * 3 * npc].rearrange(
                    "p (v c n) -> p v c n", v=nv, c=3, n=npc
                )
                t = wpool.tile([P, MAXNV * 3], BF, tag="t")
                with nc.allow_low_precision(reason="fp16 neighbor sums"):
                    nc.vector.tensor_reduce(
                        out=t[:, : nv * 3], in_=xv,
                        axis=mybir.AxisListType.X,
                        op=mybir.AluOpType.add,
                    )
                # u[p, v, k*3+j] = t[p,v,j]*lrf9[p,v,k*3+j] broadcast mul (Pool)
                u = u_bufs[c % 4]
                u9 = u[:, : nv * 16].rearrange("p (v s) -> p v s", s=16)[
                    :, :, 0:9
                ].rearrange("p v (k j) -> p v k j", k=3, j=3)
                aux9 = xsb[:, o_aux[c] : o_aux[c] + nv * 9].rearrange(
                    "p (v f) -> p v f", f=9
                )
                t4 = t[:, : nv * 3].rearrange("p (v c) -> p v c", c=3).unsqueeze(2)
                nc.gpsimd.tensor_tensor(
                    out=u9,
                    in0=t4.to_broadcast([P, nv, 3, 3]),
                    in1=aux9.rearrange("p v (k j) -> p v k j", k=3, j=3),
                    op=mybir.AluOpType.mult,
                )
                state[c] = [u, None, None]

            def stage_tu(c):
                nv = CHUNKS[c]
                cw = nv * 16
                u = state[c][0]
                pt = pst.tile([P, P], BF, tag="pt")
                nc.tensor.transpose(
                    out=pt[:cw, :], in_=u[:, :cw], identity=ident[:]
                )
                uT = wpool.tile([P, P], BF, tag="uT")
                nc.vector.tensor_copy(out=uT[:cw, :], in_=pt[:cw, :])
                state[c][1] = uT

            def stage_gemm(c):
                nv = CHUNKS[c]
                uT = state[c][1]
                pg = psg.tile([P, MAXNV * P], dt.float32, tag="pg")
                g = 0
                while g < nv:
                    ng = min(4, nv - g)
                    rb = 16 * g
                    nc.tensor.matmul(
                        out=pg[:, g * P : (g + ng) * P],
                        lhsT=uT[rb : rb + 16 * ng, :],
                        rhs=w_t[rb : rb + 16 * ng, : ng * P],
                        start=True,
                        stop=True,
                    )
                    g += ng
                state[c][2] = pg

            # store flush groups: (last chunk of group -> (first chunk, end))
            flush_of = {1: (0, 2), 3: (2, 4), 5: (4, 6), 6: (6, 7), 7: (7, 8)}

            def stage_drain_store(c):
                nv = CHUNKS[c]
                ow = nv * P
                olo = vstart[c] * P
                pg = state[c][2]
                nc.scalar.copy(out=outsb[:, olo : olo + ow], in_=pg[:, :ow])
                if c in flush_of:
                    ca, cb = flush_of[c]
                    lo = vstart[ca] * P
                    hi = vstart[cb] * P
                    eng = nc.sync if c <= 3 else nc.gpsimd
                    eng.dma_start(out=out[:, lo:hi], in_=outsb[:, lo:hi])

            # software-pipelined emission: reduce/mult 2 chunks ahead of the
            # GEMM, transpose/uT 1 ahead, drain right after its GEMM
            for i in range(NCH + 3):
                if i < NCH:
                    stage_reduce_mult(i)
                if 0 <= i - 1 < NCH:
                    stage_tu(i - 1)
                if 0 <= i - 2 < NCH:
                    stage_gemm(i - 2)
                if 0 <= i - 3 < NCH:
                    stage_drain_store(i - 3)
    return nc


def _host_prep(verts, edges, lrf, W, b):
    vb = np.asarray(verts, dtype=np.float32)
    e = np.asarray(edges).astype(np.int64)
    src = np.concatenate([e[:, 0], e[:, 1]]).astype(np.int64)
    dst = np.concatenate([e[:, 1], e[:, 0]]).astype(np.int64)

    deg = np.bincount(src, minlength=V).astype(np.int64)
    maxN = int(deg.max())

    # per-core remap: sort by degree ascending -> low-degree chunks get
    # narrow neighbor tables
    degc = deg.reshape(NCORES, VC)
    newpos = np.empty((NCORES, VC), np.int64)
    order_c = np.empty((NCORES, VC), np.int64)
    for cc in range(NCORES):
        oc = np.argsort(degc[cc], kind="stable")
        order_c[cc] = oc
        newpos[cc, oc] = np.arange(VC)

    vstart = [0]
    for nv in CHUNKS:
        vstart.append(vstart[-1] + nv)
    deg_sorted = np.sort(degc, axis=1)          # per core, ascending
    NPC = []
    for c in range(NCH):
        hi = min(vstart[c + 1] * P, VC)
        cap = int(deg_sorted[:, :hi].max())     # max over cores for SPMD
        NPC.append(cap + 1)

    order = np.argsort(src, kind="stable")
    src_s = src[order]
    dst_s = dst[order]
    starts = np.zeros(V + 1, np.int64)
    np.cumsum(deg, out=starts[1:])
    slot = np.arange(src_s.size, dtype=np.int64) - starts[src_s]

    c_a = src_s // VC
    il_new = newpos[c_a, src_s - c_a * VC]
    p_a = il_new % P
    v_a = il_new // P
    vals = vb[dst_s].astype(BF_NP)

    chunk_of_tile = np.zeros(NVT, np.int64)
    for c in range(NCH):
        chunk_of_tile[vstart[c] : vstart[c + 1]] = c
    ch_a = chunk_of_tile[v_a]

    Xp = [np.zeros((NCORES, P, CHUNKS[c], 3, NPC[c]), BF_NP) for c in range(NCH)]
    for c in range(NCH):
        m = ch_a == c
        Xp[c][c_a[m], p_a[m], v_a[m] - vstart[c], :, slot[m]] = vals[m]

    # fold slot: -deg*verts for the owned vertex goes in the last slot
    dv = (-deg[:, None].astype(np.float32)) * vb
    dv_pad = np.zeros((NCORES, VCP, 3), np.float32)
    for cc in range(NCORES):
        dv_pad[cc, :VC] = dv.reshape(NCORES, VC, 3)[cc][order_c[cc]]
    dv_t = dv_pad.reshape(NCORES, NVT, P, 3).transpose(0, 2, 1, 3)  # [NC,P,NVT,3]
    for c in range(NCH):
        Xp[c][:, :, :, :, NPC[c] - 1] = dv_t[
            :, :, vstart[c] : vstart[c + 1], :
        ].astype(BF_NP)

    # aux per vertex: lrf(9), remapped -> [NC, P, NVT, 9]
    aux_flat = np.zeros((NCORES, VCP, 9), np.float32)
    # k-major flattening: slot s = k*3+j holds lrf[:, j, k]
    lrf9 = np.ascontiguousarray(
        np.asarray(lrf, np.float32).reshape(NCORES, VC, 3, 3).transpose(0, 1, 3, 2)
    ).reshape(NCORES, VC, 9)
    for cc in range(NCORES):
        aux_flat[cc, :VC] = lrf9[cc][order_c[cc]]
    auxh = aux_flat.reshape(NCORES, NVT, P, 9).transpose(0, 2, 1, 3).astype(BF_NP)

    Wf = np.asarray(W, np.float32)
    W16 = np.zeros((16, P), np.float32)
    for s in range(9):
        W16[s, :] = Wf[:, s // 3]   # k-major: slot s = k*3+j -> k = s//3
    W16[9, :] = maxN * np.asarray(b, np.float32)
    half = np.zeros((64, 512), np.float32)
    for q in range(4):
        half[16 * q : 16 * q + 16, 128 * q : 128 * q + 128] = W16
    Wr = np.ascontiguousarray(np.vstack([half, half])).astype(BF_NP)

    in_maps = []
    for cc in range(NCORES):
        parts = []
        for gi, chs in enumerate(LGROUPS):
            for c in chs:
                parts.append(np.ascontiguousarray(Xp[c][cc].reshape(P, -1)))
            for c in chs:
                parts.append(
                    np.ascontiguousarray(
                        auxh[cc, :, vstart[c] : vstart[c + 1]].reshape(P, -1)
                    )
                )
            if gi == 0:
                parts.append(Wr)
        in_maps.append({"blob4": np.ascontiguousarray(np.concatenate(parts, axis=1))})
    return in_maps, NPC, order_c


def kernel(verts, edges, lrf, W, b):
    global LAST_RESULTS
    in_maps, NPC, order_c = _host_prep(verts, edges, lrf, W, b)

    nc = bacc.Bacc()
    build(nc, NPC)
    nc.finalize()

    trace = os.environ.get("KBENCH_TRACE") == "1"
    res = run_bass_kernel_spmd(
        nc, in_maps, core_ids=list(range(NCORES)), trace=trace
    )
    LAST_RESULTS = res

    full = np.empty((V, 128), np.float32)
    for c in range(NCORES):
        o = (
            res.results[c]["out"].astype(np.float32)
            .reshape(P, NVT, P).transpose(1, 0, 2).reshape(VCP, P)[:VC]
        )
        blk = full[c * VC : (c + 1) * VC]
        blk[order_c[c]] = o
    return full
